# revision 7
# baseline (speedup 1.0000x reference)
"""Trainium2 Bass kernel for the reference MultiHeadAttention module.

Problem: B=32, T=512, D=1024, H=16, HD=64.

Reference computation (unusual orientation: keys index rows, queries
index softmax axis, no 1/sqrt(d) scale):
    h  = x @ Wi + bi
    k/q/v = per-head h @ W{k,q,v}[h] + b (head-stacked weights)
    wei[b,h,t,s] = k[b,h,t,:] . q[b,h,s,:]      (t = key idx, s = query idx)
    wei masked to s <= t, softmax over s
    out = (wei @ v) concat-heads @ Wo + bo

Host-side algebra: h is ONLY consumed by the q/k/v projections, so Wi
folds into them (Wq' = Wi @ Wq, float64 accum) and the device never
computes h; biases fold likewise (bv through Wo into a constant row).

Sharding: data-parallel over batch, 4 batches per core, replicated
weights, no collectives.

HW-measured engine budget drives the design (per-core, micro-benched):
f32r matmul N=512 ~94-134 ns; DVE [128,512] PSUM->SBUF copy ~743 ns
(PSUM reads are 4x slower than SBUF); ACT exp ~662 ns. The kernel time
tracks the SUM of engine busy times, so work is balanced across engines:
  - causal mask applied ON THE PE (tri accumulated into the S psum via
    an extra matmul: triT^T @ I) instead of DVE tensor_add; this also
    removes a PE->DVE->ACT hop from the S->exp chain,
  - phase-A PSUM->SBUF copies alternate between DVE and ACT (AF.Copy),
  - softmax normalization (reciprocal/broadcast/multiply) is emitted one
    head-pair late so the DVE FIFO head never waits on an unfinished PV
    matmul (head-of-line blocking),
  - out_proj tiles of the previous batch are zipped between the S and PV
    stages of the current pair, giving exp a 2-stage slack window.

Phase A per 512-token chunk: q^T/k^T (feature-major) and v (token-major,
[v|1] per head) from host-transposed x^T; batch 0 stays SBUF-resident,
batches 1-3 spill to DRAM (DMA is fully hidden: measured compute-only
time == full-kernel time).

Phase B per batch: S^T[s,t] per head via q^T/k^T slices (base partitions
0/64), PE-side mask, P^T = exp(S^T) on ACT (no max-subtraction: unscaled
reference softmax keeps |logits| <= ~55, safe in fp32 and errors cancel
in the ratio), O^T = [V|1]^T @ P^T (ones column = denominator row 64;
for the last s-tile the mask extends over [256:384] so PV streams >=256
cols, f32r full-rate), normalize, out = O^T-chain @ Wo.
"""

import sys

sys.path.insert(0, "/opt/trn_rl_repo")

import numpy as np

import concourse.bacc as bacc
import concourse.mybir as mybir
from concourse import bass_utils
from concourse.tile import TileContext

F32 = mybir.dt.float32
F32R = mybir.dt.float32r
BF16 = mybir.dt.bfloat16
AF = mybir.ActivationFunctionType

B, T, D, H, HD = 32, 512, 1024, 16, 64
NCORES = 8
BN = B // NCORES          # batches per core = 4
TOK = BN * T              # tokens per core = 2048
NKT = D // 128            # 8 contraction tiles
NMC = TOK // 512          # 4 token chunks (phase A)
MASK_NEG = -60000.0       # exp(-60000 + |logit|) == 0 in fp32

_CACHE = {}


def _build(with_qk_bias: bool, repeat: int = 1):
    nc = bacc.Bacc("TRN2", target_bir_lowering=False, debug=False,
                   num_devices=NCORES)

    xT = nc.dram_tensor("xT", [D, TOK], F32, kind="ExternalInput")
    wq = nc.dram_tensor("wq", [D, D], F32, kind="ExternalInput")
    wk = nc.dram_tensor("wk", [D, D], F32, kind="ExternalInput")
    wv = nc.dram_tensor("wv", [D, D], F32, kind="ExternalInput")
    wo = nc.dram_tensor("wo", [D, D], F32, kind="ExternalInput")
    triT = nc.dram_tensor("triT", [128, 128], F32, kind="ExternalInput")
    ident = nc.dram_tensor("ident", [128, 128], F32, kind="ExternalInput")
    onesr = nc.dram_tensor("onesr", [1, 128], F32, kind="ExternalInput")
    negrow = nc.dram_tensor("negrow", [1, 128], F32, kind="ExternalInput")
    onesc = nc.dram_tensor("onesc", [128, H], F32, kind="ExternalInput")
    if with_qk_bias:
        bq2 = nc.dram_tensor("bq2", [128, NKT], F32, kind="ExternalInput")
        bk2 = nc.dram_tensor("bk2", [128, NKT], F32, kind="ExternalInput")
    out = nc.dram_tensor("out", [TOK, D], F32, kind="ExternalOutput")

    # DRAM scratch spills for batches 1..3 (feature-major q/k, token-major v)
    qT_d = nc.dram_tensor("qT_d", [D, TOK], F32, kind="Internal")
    kT_d = nc.dram_tensor("kT_d", [D, TOK], F32, kind="Internal")
    v_d = nc.dram_tensor("v_d", [TOK, H * 65], F32, kind="Internal")

    with TileContext(nc) as tc:
      for _rep in range(repeat):
        with tc.tile_pool(name="const", bufs=1) as cpool:
            triT_f = cpool.tile([128, 128], F32, tag="triTf")
            nc.sync.dma_start(triT_f[:], triT[:])
            ident_f = cpool.tile([128, 128], F32, tag="identf")
            nc.sync.dma_start(ident_f[:], ident[:])
            onesr_f = cpool.tile([1, 128], F32, tag="onesrf")
            nc.sync.dma_start(onesr_f[:], onesr[:])
            negrow_f = cpool.tile([1, 128], F32, tag="negrowf")
            nc.sync.dma_start(negrow_f[:], negrow[:])
            triT_sb = cpool.tile([128, 128], BF16, tag="triT")
            nc.vector.tensor_copy(triT_sb[:], triT_f[:])
            ident_sb = cpool.tile([128, 128], BF16, tag="ident")
            nc.vector.tensor_copy(ident_sb[:], ident_f[:])
            onesr_sb = cpool.tile([1, 128], BF16, tag="onesr")
            nc.vector.tensor_copy(onesr_sb[:], onesr_f[:])
            negrow_sb = cpool.tile([1, 128], BF16, tag="negrow")
            nc.vector.tensor_copy(negrow_sb[:], negrow_f[:])
            ones16_sb = cpool.tile([128, H], F32, tag="ones16")
            nc.sync.dma_start(ones16_sb[:], onesc[:])
            if with_qk_bias:
                bq_sb = cpool.tile([128, NKT], F32, tag="bq")
                bk_sb = cpool.tile([128, NKT], F32, tag="bk")
                nc.sync.dma_start(bq_sb[:], bq2[:])
                nc.sync.dma_start(bk_sb[:], bk2[:])

            qkpool_cm = tc.tile_pool(name="qk", bufs=1)
            qkpool = qkpool_cm.__enter__()
            # batch-0 q^T/k^T and all vp tiles live here (shared across
            # phases; batches 1-3 reload into the same tags)
            qt0 = [qkpool.tile([128, 512], F32R, tag=f"qt{e}", bufs=1, name=f"qt{e}")
                   for e in range(NKT)]
            kt0 = [qkpool.tile([128, 512], F32R, tag=f"kt{e}", bufs=1, name=f"kt{e}")
                   for e in range(NKT)]
            vp0 = [qkpool.tile([128, H * 65], F32R, tag=f"vp{i}", bufs=2, name=f"vp{i}")
                   for i in range(4)]

            # alternate PSUM->SBUF copies between DVE and ACT to balance
            # engine load (both measured ~700 ns per [128,512] tile)
            cp_flip = [0]

            def ps_copy(dst_ap, src_ap):
                cp_flip[0] ^= 1
                if cp_flip[0]:
                    nc.vector.tensor_copy(dst_ap, src_ap)
                else:
                    nc.scalar.activation(dst_ap, src_ap, AF.Copy)

            # ---------------- Phase A: QKV projections ----------------
            with tc.tile_pool(name="wA", bufs=1) as wpool, \
                 tc.tile_pool(name="actA", bufs=1) as apool, \
                 tc.tile_pool(name="psA", bufs=1, space="PSUM") as pspool:
                wq_sb = [wpool.tile([128, D], F32R, tag=f"wq{k}", name=f"wq{k}") for k in range(NKT)]
                wk_sb = [wpool.tile([128, D], F32R, tag=f"wk{k}", name=f"wk{k}") for k in range(NKT)]
                wv_sb = [wpool.tile([128, D], F32R, tag=f"wv{k}", name=f"wv{k}") for k in range(NKT)]
                # Startup critical path: chunk 0 needs Wq + x first, so
                # interleave those DMAs; Wk/Wv stream in under chunk-0's
                # q-projection.
                xc0 = [apool.tile([128, 512], F32R, tag=f"xc{k}", bufs=1, name=f"xc{k}")
                       for k in range(NKT)]
                for k in range(NKT):
                    nc.sync.dma_start(wq_sb[k][:], wq[128 * k:128 * (k + 1), :].bitcast(F32R))
                    nc.sync.dma_start(
                        xc0[k][:], xT[128 * k:128 * (k + 1), 0:512].bitcast(F32R))
                for k in range(NKT):
                    nc.sync.dma_start(wk_sb[k][:], wk[128 * k:128 * (k + 1), :].bitcast(F32R))
                for k in range(NKT):
                    nc.sync.dma_start(wv_sb[k][:], wv[128 * k:128 * (k + 1), :].bitcast(F32R))

                for mc in range(NMC):
                    c0 = 512 * mc
                    if mc == 0:
                        xc = xc0
                    else:
                        xc = [apool.tile([128, 512], F32R, tag=f"xc{k}", bufs=1, name=f"xc{k}")
                              for k in range(NKT)]
                        for k in range(NKT):
                            nc.sync.dma_start(
                                xc[k][:], xT[128 * k:128 * (k + 1), c0:c0 + 512].bitcast(F32R))

                    # q^T / k^T chunks (feature-major); chunk 0 direct to
                    # SBUF, others spilled
                    for w_sb, b_ap, dst, sb0 in (
                        (wq_sb, "bq", qT_d, qt0), (wk_sb, "bk", kT_d, kt0)):
                        for n in range(NKT):
                            pq = pspool.tile([128, 512], F32, tag="ps", bufs=6)
                            for k in range(NKT):
                                nc.tensor.matmul(
                                    pq[:], w_sb[k][:, 128 * n:128 * (n + 1)], xc[k][:],
                                    start=(k == 0), stop=(k == NKT - 1))
                            if mc == 0:
                                qs_t = sb0[n][:]
                            else:
                                qs = apool.tile([128, 512], F32, tag="spill", bufs=4,
                                                name=f"qs{n}")
                                qs_t = qs[:]
                            if with_qk_bias:
                                bias = (bq_sb if b_ap == "bq" else bk_sb)[:, n:n + 1]
                                nc.vector.tensor_scalar_add(qs_t, pq[:], bias)
                            else:
                                ps_copy(qs_t, pq[:])
                            if mc != 0:
                                nc.sync.dma_start(
                                    dst[128 * n:128 * (n + 1), c0:c0 + 512], qs_t)

                    # v chunk (token-major), staged into v_plus layout
                    # ([v|1] per head); chunk 0 direct to vp tiles
                    for tt in range(4):
                        if mc == 0:
                            vs2 = None
                            v3 = vp0[tt][:].rearrange("p (h e) -> p h e", e=65)
                        else:
                            vs2 = apool.tile([128, H * 65], F32, tag="vsp", bufs=2,
                                             name=f"vs2{tt}")
                            v3 = vs2[:].rearrange("p (h e) -> p h e", e=65)
                        pv = [pspool.tile([128, 512], F32, tag="psv", bufs=2,
                                          name=f"psv{nn}") for nn in range(2)]
                        for k in range(NKT):
                            for nn in range(2):
                                nc.tensor.matmul(
                                    pv[nn][:], xc[k][:, 128 * tt:128 * (tt + 1)],
                                    wv_sb[k][:, 512 * nn:512 * (nn + 1)],
                                    start=(k == 0), stop=(k == NKT - 1))
                        for nn in range(2):
                            ps_copy(v3[:, 8 * nn:8 * (nn + 1), 0:64],
                                    pv[nn][:].rearrange("p (h e) -> p h e", e=64))
                        nc.vector.tensor_copy(v3[:, :, 64], ones16_sb[:])
                        if mc != 0:
                            nc.sync.dma_start(
                                v_d[c0 + 128 * tt:c0 + 128 * (tt + 1), :], vs2[:])

            # ---------------- Phase B: attention + out_proj per batch --
            with tc.tile_pool(name="wB", bufs=1) as wpool, \
                 tc.tile_pool(name="actB", bufs=1) as apool, \
                 tc.tile_pool(name="psB", bufs=1, space="PSUM") as psB:
                wo_sb = [wpool.tile([128, D], F32R, tag=f"wo{k}", name=f"wo{k}") for k in range(NKT)]
                for k in range(NKT):
                    nc.sync.dma_start(wo_sb[k][:], wo[128 * k:128 * (k + 1), :].bitcast(F32R))

                def qkv_loads(b):
                    """Spill reloads for batch b (batch 0 is resident)."""
                    r0 = 512 * b
                    if b == 0:
                        return qt0, kt0, vp0
                    qt = [qkpool.tile([128, 512], F32R, tag=f"qt{e}", bufs=1, name=f"qt{e}")
                          for e in range(NKT)]
                    kt = [qkpool.tile([128, 512], F32R, tag=f"kt{e}", bufs=1, name=f"kt{e}")
                          for e in range(NKT)]
                    for e in range(NKT):
                        nc.sync.dma_start(
                            qt[e][:], qT_d[128 * e:128 * (e + 1), r0:r0 + 512].bitcast(F32R))
                        nc.sync.dma_start(
                            kt[e][:], kT_d[128 * e:128 * (e + 1), r0:r0 + 512].bitcast(F32R))
                    vp = [qkpool.tile([128, H * 65], F32R, tag=f"vp{i}", bufs=2, name=f"vp{i}")
                          for i in range(4)]
                    for i in range(4):
                        nc.sync.dma_start(
                            vp[i][:],
                            v_d[r0 + 128 * i:r0 + 128 * (i + 1), :].bitcast(F32R))
                    return qt, kt, vp

                def smme(p, st):
                    """S matmuls + PE-side mask + exp for global pair p."""
                    qt, kt, _ = st["qkv"]
                    m = p % 8
                    pts = {}
                    for i in range(4):
                        w0 = 128 * i  # valid t-cols are [w0, 512)
                        wS = min(w0, 256)  # keep moving dim >= 256 (f32r)
                        for j in (2 * m, 2 * m + 1):
                            off = 64 * (j % 2)
                            ps = psB.tile([128, 512], F32, tag="ps", bufs=4,
                                          name=f"ps{i}{j % 2}")
                            nc.tensor.matmul(
                                ps[:, wS:512],
                                qt[m][off:off + 64, w0:w0 + 128],
                                kt[m][off:off + 64, wS:512],
                                start=True, stop=False)
                            # causal mask on the PE: accumulate triT^T @ I
                            # into the diagonal block (bf16 operands: mask
                            # values are bf16-exact, full rate at any free
                            # dim); exp/PV only consume [w0:512)
                            nc.tensor.matmul(
                                ps[:, w0:w0 + 128], triT_sb[:], ident_sb[:],
                                start=False, stop=True, skip_group_check=True)
                            pt = apool.tile([128, 512], F32R, tag="pt", bufs=18,
                                            name=f"pt{i}{j % 2}")
                            nc.scalar.activation(pt[:, w0:512], ps[:, w0:512], AF.Exp)
                            pts[(j % 2, i)] = pt
                    st["pts"][p] = pts

                def pv_stage(p, st):
                    """PV accumulation for global pair p."""
                    _, _, vp = st["qkv_of"][p]
                    m = p % 8
                    pts = st["pts"].pop(p)
                    pos = {}
                    for jj in (0, 1):
                        j = 2 * m + jj
                        po = psB.tile([65, 512], F32, tag="po", bufs=2, name=f"po{jj}")
                        pos[jj] = po
                        for i in range(4):
                            w0 = 128 * i
                            nc.tensor.matmul(
                                po[0:65, w0:512],
                                vp[i][:, 65 * j:65 * (j + 1)],
                                pts[(jj, i)][:, w0:512],
                                start=(i == 0), stop=(i == 3), skip_group_check=True)
                    st["pos"][p] = pos

                def norm_stage(p, st):
                    """Normalize pair p into its oT tiles (one slot late so
                    the DVE FIFO never heads-of-line blocks on PV)."""
                    m = p % 8
                    oT = st["oT_of"][p]
                    pos = st["pos"].pop(p)
                    for jj in (0, 1):
                        off = 64 * jj
                        rs = apool.tile([1, 512], F32R, tag="rs", bufs=4, name="rs")
                        with nc.allow_low_precision(reason="f32r softmax recip"):
                            nc.vector.reciprocal(rs[:], pos[jj][64:65, :])
                        rb = apool.tile([64, 512], F32R, tag="rb", bufs=4, name="rb")
                        nc.gpsimd.partition_broadcast(rb[:], rs[:])
                        nc.vector.tensor_mul(oT[m][off:off + 64, :],
                                             pos[jj][0:64, :], rb[:])

                def op_unit(u, st):
                    """Both nn halves of out_proj tile tt (shared oT
                    stationaries: half the LDWEIGHTS)."""
                    b_op, tt = divmod(u, 4)
                    oT = st["oT_b"][b_op]
                    r0 = 512 * b_op
                    pf = [psB.tile([128, 512], F32, tag="pf", bufs=2,
                                   name=f"pf{nn}") for nn in range(2)]
                    for k in range(NKT):
                        for nn in range(2):
                            nc.tensor.matmul(
                                pf[nn][:], oT[k][:, 128 * tt:128 * (tt + 1)],
                                wo_sb[k][:, 512 * nn:512 * (nn + 1)],
                                start=(k == 0), stop=(k == NKT - 1))
                    for nn in range(2):
                        os_ = apool.tile([128, 512], F32, tag="os", bufs=3,
                                         name=f"os{tt}{nn}")
                        ps_copy(os_[:], pf[nn][:])
                        nc.sync.dma_start(
                            out[r0 + 128 * tt:r0 + 128 * (tt + 1),
                                512 * nn:512 * (nn + 1)], os_[:])

                NP = BN * (H // 2)  # 32 global pairs
                st = {"pts": {}, "pos": {}, "qkv_of": {}, "oT_of": {},
                      "oT_b": {}, "qkv": None}
                for p in range(NP + 10):
                    b, m = divmod(p, 8)
                    if p < NP:
                        if m == 0:
                            st["qkv"] = qkv_loads(b)
                            st["oT_b"][b] = [
                                apool.tile([128, 512], F32R, tag=f"oT{e}", bufs=2,
                                           name=f"oT{e}") for e in range(NKT)]
                        st["qkv_of"][p] = st["qkv"]
                        st["oT_of"][p] = st["oT_b"][b]
                        smme(p, st)
                    if p - 10 >= 0 and p - 10 < NP and (p - 10) % 2 == 0:
                        op_unit((p - 10) // 2, st)
                    if p - 1 >= 0 and p - 1 < NP:
                        pv_stage(p - 1, st)
                    if p - 2 >= 0 and p - 2 < NP:
                        norm_stage(p - 2, st)
            qkpool_cm.__exit__(None, None, None)

    nc.compile()
    return nc


def _ensure_built(with_qk_bias: bool, repeat: int = 1):
    key = (with_qk_bias, repeat)
    if key not in _CACHE:
        _CACHE[key] = _build(with_qk_bias, repeat)
    return _CACHE[key]


def _prepare(x, Wi, bi, Wk, bk, Wq, bq, Wv, bv, Wo, bo):
    """Host-side prep: returns (in_maps, out_const, with_qk_bias)."""
    x, Wi, bi = np.asarray(x, np.float32), np.asarray(Wi, np.float32), np.asarray(bi, np.float32)
    Wk, bk = np.asarray(Wk, np.float32), np.asarray(bk, np.float32)
    Wq, bq = np.asarray(Wq, np.float32), np.asarray(bq, np.float32)
    Wv, bv = np.asarray(Wv, np.float32), np.asarray(bv, np.float32)
    Wo, bo = np.asarray(Wo, np.float32), np.asarray(bo, np.float32)

    # flatten head-stacked weights: col f = h*HD + e
    wq_f = np.ascontiguousarray(Wq.transpose(1, 0, 2).reshape(D, D))
    wk_f = np.ascontiguousarray(Wk.transpose(1, 0, 2).reshape(D, D))
    wv_f = np.ascontiguousarray(Wv.transpose(1, 0, 2).reshape(D, D))
    # fold the in_proj into the q/k/v weights (h is only consumed by them)
    Wi64 = Wi.astype(np.float64)
    wq_c = (Wi64 @ wq_f.astype(np.float64)).astype(np.float32)
    wk_c = (Wi64 @ wk_f.astype(np.float64)).astype(np.float32)
    wv_c = (Wi64 @ wv_f.astype(np.float64)).astype(np.float32)
    # fold bi through the qkv projections; fold bv through out_proj
    bq_fold = (bi @ wq_f + bq.reshape(-1)).astype(np.float32)
    bk_fold = (bi @ wk_f + bk.reshape(-1)).astype(np.float32)
    bv_fold = (bi @ wv_f + bv.reshape(-1)).astype(np.float32)
    out_const = (bv_fold @ Wo + bo).astype(np.float32)  # added host-side

    with_qk_bias = bool(np.any(bq_fold) or np.any(bk_fold))

    # additive causal mask for the [s,t] diagonal block: -60000 where s > t
    tri_add = ((np.triu(np.ones((128, 128))) - 1.0) * -MASK_NEG).astype(np.float32)
    onesc = np.ones((128, H), np.float32)

    shared = {"wq": wq_c, "wk": wk_c, "wv": wv_c, "wo": Wo,
              "triT": np.ascontiguousarray(tri_add.T),
              "ident": np.eye(128, dtype=np.float32),
              "onesr": np.ones((1, 128), np.float32),
              "negrow": np.full((1, 128), MASK_NEG, np.float32),
              "onesc": onesc}
    if with_qk_bias:
        shared["bq2"] = np.ascontiguousarray(bq_fold.reshape(NKT, 128).T)
        shared["bk2"] = np.ascontiguousarray(bk_fold.reshape(NKT, 128).T)

    in_maps = []
    for c in range(NCORES):
        xs = x[BN * c:BN * (c + 1)].reshape(TOK, D)
        m = dict(shared)
        m["xT"] = np.ascontiguousarray(xs.T)
        in_maps.append(m)
    return in_maps, out_const, with_qk_bias


def kernel(x, Wi, bi, Wk, bk, Wq, bq, Wv, bv, Wo, bo):
    in_maps, out_const, with_qk_bias = _prepare(
        x, Wi, bi, Wk, bk, Wq, bq, Wv, bv, Wo, bo)
    nc = _ensure_built(with_qk_bias)
    res = bass_utils.run_bass_kernel_spmd(nc, in_maps, core_ids=list(range(NCORES)))
    outs = [res.results[c]["out"] for c in range(NCORES)]
    full = np.concatenate(outs, axis=0).reshape(B, T, D)
    full += out_const[None, None, :]
    return full


# revision 8
# speedup vs baseline: 1.2442x; 1.2442x over previous
"""Trainium2 Bass kernel for the reference MultiHeadAttention module.

Problem: B=32, T=512, D=1024, H=16, HD=64.

Reference computation (unusual orientation: keys index rows, queries
index softmax axis, no 1/sqrt(d) scale):
    h  = x @ Wi + bi
    k/q/v = per-head h @ W{k,q,v}[h] + b (head-stacked weights)
    wei[b,h,t,s] = k[b,h,t,:] . q[b,h,s,:]      (t = key idx, s = query idx)
    wei masked to s <= t, softmax over s
    out = (wei @ v) concat-heads @ Wo + bo

Host-side algebra: h is ONLY consumed by the q/k/v projections, so Wi
folds into them (Wq' = Wi @ Wq, float64 accum) and the device never
computes h; biases fold likewise (bv through Wo into a constant row).

Sharding: data-parallel over batch, 4 batches per core, replicated
weights, no collectives.

HW-measured engine budget drives the design (per-core, micro-benched):
f32r matmul N=512 ~94-134 ns; DVE [128,512] PSUM->SBUF copy ~743 ns
(PSUM reads are 4x slower than SBUF); ACT exp ~662 ns. The kernel time
tracks the SUM of engine busy times, so work is balanced across engines:
  - causal mask applied ON THE PE (tri accumulated into the S psum via
    an extra matmul: triT^T @ I) instead of DVE tensor_add; this also
    removes a PE->DVE->ACT hop from the S->exp chain,
  - phase-A PSUM->SBUF copies alternate between DVE and ACT (AF.Copy),
  - softmax normalization (reciprocal/broadcast/multiply) is emitted one
    head-pair late so the DVE FIFO head never waits on an unfinished PV
    matmul (head-of-line blocking),
  - out_proj tiles of the previous batch are zipped between the S and PV
    stages of the current pair, giving exp a 2-stage slack window.

Phase A per 512-token chunk: q^T/k^T (feature-major) and v (token-major,
[v|1] per head) from host-transposed x^T; batch 0 stays SBUF-resident,
batches 1-3 spill to DRAM (DMA is fully hidden: measured compute-only
time == full-kernel time).

Phase B per batch: S^T[s,t] per head via q^T/k^T slices (base partitions
0/64), PE-side mask, P^T = exp(S^T) on ACT (no max-subtraction: unscaled
reference softmax keeps |logits| <= ~55, safe in fp32 and errors cancel
in the ratio), O^T = [V|1]^T @ P^T (ones column = denominator row 64;
for the last s-tile the mask extends over [256:384] so PV streams >=256
cols, f32r full-rate), normalize, out = O^T-chain @ Wo.
"""

import sys

sys.path.insert(0, "/opt/trn_rl_repo")

import numpy as np

import concourse.bacc as bacc
import concourse.mybir as mybir
from concourse import bass_utils
from concourse.tile import TileContext

F32 = mybir.dt.float32
F32R = mybir.dt.float32r
BF16 = mybir.dt.bfloat16
AF = mybir.ActivationFunctionType

B, T, D, H, HD = 32, 512, 1024, 16, 64
NCORES = 8
BN = B // NCORES          # batches per core = 4
TOK = BN * T              # tokens per core = 2048
NKT = D // 128            # 8 contraction tiles
NMC = TOK // 512          # 4 token chunks (phase A)
MASK_NEG = -60000.0       # exp(-60000 + |logit|) == 0 in fp32

_CACHE = {}


def _build(with_qk_bias: bool, repeat: int = 1):
    nc = bacc.Bacc("TRN2", target_bir_lowering=False, debug=False,
                   num_devices=NCORES)

    xT = nc.dram_tensor("xT", [D, TOK], F32, kind="ExternalInput")
    wq = nc.dram_tensor("wq", [D, D], F32, kind="ExternalInput")
    wk = nc.dram_tensor("wk", [D, D], F32, kind="ExternalInput")
    wv = nc.dram_tensor("wv", [D, D], F32, kind="ExternalInput")
    wo = nc.dram_tensor("wo", [D, D], F32, kind="ExternalInput")
    triT = nc.dram_tensor("triT", [128, 128], F32, kind="ExternalInput")
    ident = nc.dram_tensor("ident", [128, 128], F32, kind="ExternalInput")
    onesr = nc.dram_tensor("onesr", [1, 128], F32, kind="ExternalInput")
    negrow = nc.dram_tensor("negrow", [1, 128], F32, kind="ExternalInput")
    onesc = nc.dram_tensor("onesc", [128, H], F32, kind="ExternalInput")
    if with_qk_bias:
        bq2 = nc.dram_tensor("bq2", [128, NKT], F32, kind="ExternalInput")
        bk2 = nc.dram_tensor("bk2", [128, NKT], F32, kind="ExternalInput")
    out = nc.dram_tensor("out", [TOK, D], F32, kind="ExternalOutput")

    # DRAM scratch spills for batches 1..3 (feature-major q/k, token-major v)
    qT_d = nc.dram_tensor("qT_d", [D, TOK], F32, kind="Internal")
    kT_d = nc.dram_tensor("kT_d", [D, TOK], F32, kind="Internal")
    v_d = nc.dram_tensor("v_d", [TOK, H * 65], F32, kind="Internal")

    with TileContext(nc) as tc:
      for _rep in range(repeat):
        with tc.tile_pool(name="const", bufs=1) as cpool:
            triT_f = cpool.tile([128, 128], F32, tag="triTf")
            nc.sync.dma_start(triT_f[:], triT[:])
            ident_f = cpool.tile([128, 128], F32, tag="identf")
            nc.sync.dma_start(ident_f[:], ident[:])
            onesr_f = cpool.tile([1, 128], F32, tag="onesrf")
            nc.sync.dma_start(onesr_f[:], onesr[:])
            negrow_f = cpool.tile([1, 128], F32, tag="negrowf")
            nc.sync.dma_start(negrow_f[:], negrow[:])
            triT_sb = cpool.tile([128, 128], BF16, tag="triT")
            nc.vector.tensor_copy(triT_sb[:], triT_f[:])
            ident_sb = cpool.tile([128, 128], BF16, tag="ident")
            nc.vector.tensor_copy(ident_sb[:], ident_f[:])
            onesr_sb = cpool.tile([1, 128], BF16, tag="onesr")
            nc.vector.tensor_copy(onesr_sb[:], onesr_f[:])
            negrow_sb = cpool.tile([1, 128], BF16, tag="negrow")
            nc.vector.tensor_copy(negrow_sb[:], negrow_f[:])
            ones16_sb = cpool.tile([128, H], F32, tag="ones16")
            nc.sync.dma_start(ones16_sb[:], onesc[:])
            if with_qk_bias:
                bq_sb = cpool.tile([128, NKT], F32, tag="bq")
                bk_sb = cpool.tile([128, NKT], F32, tag="bk")
                nc.sync.dma_start(bq_sb[:], bq2[:])
                nc.sync.dma_start(bk_sb[:], bk2[:])

            qkpool_cm = tc.tile_pool(name="qk", bufs=1)
            qkpool = qkpool_cm.__enter__()
            # batch-0 q^T/k^T and all vp tiles live here (shared across
            # phases; batches 1-3 reload into the same tags)
            qt0 = [qkpool.tile([128, 512], F32R, tag=f"qt{e}", bufs=1, name=f"qt{e}")
                   for e in range(NKT)]
            kt0 = [qkpool.tile([128, 512], F32R, tag=f"kt{e}", bufs=1, name=f"kt{e}")
                   for e in range(NKT)]
            vp0 = [qkpool.tile([128, H * 65], F32R, tag=f"vp{i}", bufs=2, name=f"vp{i}")
                   for i in range(4)]

            # alternate PSUM->SBUF copies between DVE and ACT to balance
            # engine load (both measured ~700 ns per [128,512] tile)
            cp_flip = [0]

            def ps_copy(dst_ap, src_ap):
                cp_flip[0] ^= 1
                if cp_flip[0]:
                    nc.vector.tensor_copy(dst_ap, src_ap)
                else:
                    nc.scalar.activation(dst_ap, src_ap, AF.Copy)

            # ---------------- Phase A: QKV projections ----------------
            with tc.tile_pool(name="wA", bufs=1) as wpool, \
                 tc.tile_pool(name="actA", bufs=1) as apool, \
                 tc.tile_pool(name="psA", bufs=1, space="PSUM") as pspool:
                wq_sb = [wpool.tile([128, D], F32R, tag=f"wq{k}", name=f"wq{k}") for k in range(NKT)]
                wk_sb = [wpool.tile([128, D], F32R, tag=f"wk{k}", name=f"wk{k}") for k in range(NKT)]
                wv_sb = [wpool.tile([128, D], F32R, tag=f"wv{k}", name=f"wv{k}") for k in range(NKT)]
                # Startup critical path: chunk 0 needs Wq + x first, so
                # interleave those DMAs; Wk/Wv stream in under chunk-0's
                # q-projection.
                xc0 = [apool.tile([128, 512], F32R, tag=f"xc{k}", bufs=1, name=f"xc{k}")
                       for k in range(NKT)]
                for k in range(NKT):
                    nc.sync.dma_start(wq_sb[k][:], wq[128 * k:128 * (k + 1), :].bitcast(F32R))
                    nc.sync.dma_start(
                        xc0[k][:], xT[128 * k:128 * (k + 1), 0:512].bitcast(F32R))
                for k in range(NKT):
                    nc.sync.dma_start(wk_sb[k][:], wk[128 * k:128 * (k + 1), :].bitcast(F32R))
                for k in range(NKT):
                    nc.sync.dma_start(wv_sb[k][:], wv[128 * k:128 * (k + 1), :].bitcast(F32R))

                for mc in range(NMC):
                    c0 = 512 * mc
                    if mc == 0:
                        xc = xc0
                    else:
                        xc = [apool.tile([128, 512], F32R, tag=f"xc{k}", bufs=1, name=f"xc{k}")
                              for k in range(NKT)]
                        for k in range(NKT):
                            nc.sync.dma_start(
                                xc[k][:], xT[128 * k:128 * (k + 1), c0:c0 + 512].bitcast(F32R))

                    # q^T / k^T chunks (feature-major); chunk 0 direct to
                    # SBUF, others spilled
                    for w_sb, b_ap, dst, sb0 in (
                        (wq_sb, "bq", qT_d, qt0), (wk_sb, "bk", kT_d, kt0)):
                        for n in range(NKT):
                            pq = pspool.tile([128, 512], F32, tag="ps", bufs=6)
                            for k in range(NKT):
                                nc.tensor.matmul(
                                    pq[:], w_sb[k][:, 128 * n:128 * (n + 1)], xc[k][:],
                                    start=(k == 0), stop=(k == NKT - 1))
                            if mc == 0:
                                qs_t = sb0[n][:]
                            else:
                                qs = apool.tile([128, 512], F32, tag="spill", bufs=4,
                                                name=f"qs{n}")
                                qs_t = qs[:]
                            if with_qk_bias:
                                bias = (bq_sb if b_ap == "bq" else bk_sb)[:, n:n + 1]
                                nc.vector.tensor_scalar_add(qs_t, pq[:], bias)
                            else:
                                ps_copy(qs_t, pq[:])
                            if mc != 0:
                                nc.sync.dma_start(
                                    dst[128 * n:128 * (n + 1), c0:c0 + 512], qs_t)

                    # v chunk (token-major), staged into v_plus layout
                    # ([v|1] per head); chunk 0 direct to vp tiles
                    for tt in range(4):
                        if mc == 0:
                            vs2 = None
                            v3 = vp0[tt][:].rearrange("p (h e) -> p h e", e=65)
                        else:
                            vs2 = apool.tile([128, H * 65], F32, tag="vsp", bufs=2,
                                             name=f"vs2{tt}")
                            v3 = vs2[:].rearrange("p (h e) -> p h e", e=65)
                        for nn in range(2):
                            pv = pspool.tile([128, 512], F32, tag="psv", bufs=2)
                            for k in range(NKT):
                                nc.tensor.matmul(
                                    pv[:], xc[k][:, 128 * tt:128 * (tt + 1)],
                                    wv_sb[k][:, 512 * nn:512 * (nn + 1)],
                                    start=(k == 0), stop=(k == NKT - 1))
                            ps_copy(v3[:, 8 * nn:8 * (nn + 1), 0:64],
                                    pv[:].rearrange("p (h e) -> p h e", e=64))
                        nc.vector.tensor_copy(v3[:, :, 64], ones16_sb[:])
                        if mc != 0:
                            nc.sync.dma_start(
                                v_d[c0 + 128 * tt:c0 + 128 * (tt + 1), :], vs2[:])

            # ---------------- Phase B: attention + out_proj per batch --
            with tc.tile_pool(name="wB", bufs=1) as wpool, \
                 tc.tile_pool(name="actB", bufs=1) as apool, \
                 tc.tile_pool(name="psB", bufs=1, space="PSUM") as psB:
                wo_sb = [wpool.tile([128, D], F32R, tag=f"wo{k}", name=f"wo{k}") for k in range(NKT)]
                for k in range(NKT):
                    nc.sync.dma_start(wo_sb[k][:], wo[128 * k:128 * (k + 1), :].bitcast(F32R))

                def qkv_loads(b):
                    """Spill reloads for batch b (batch 0 is resident)."""
                    r0 = 512 * b
                    if b == 0:
                        return qt0, kt0, vp0
                    qt = [qkpool.tile([128, 512], F32R, tag=f"qt{e}", bufs=1, name=f"qt{e}")
                          for e in range(NKT)]
                    kt = [qkpool.tile([128, 512], F32R, tag=f"kt{e}", bufs=1, name=f"kt{e}")
                          for e in range(NKT)]
                    for e in range(NKT):
                        nc.sync.dma_start(
                            qt[e][:], qT_d[128 * e:128 * (e + 1), r0:r0 + 512].bitcast(F32R))
                        nc.sync.dma_start(
                            kt[e][:], kT_d[128 * e:128 * (e + 1), r0:r0 + 512].bitcast(F32R))
                    vp = [qkpool.tile([128, H * 65], F32R, tag=f"vp{i}", bufs=2, name=f"vp{i}")
                          for i in range(4)]
                    for i in range(4):
                        nc.sync.dma_start(
                            vp[i][:],
                            v_d[r0 + 128 * i:r0 + 128 * (i + 1), :].bitcast(F32R))
                    return qt, kt, vp

                def smme(p, st):
                    """S matmuls + PE-side mask + exp for global pair p."""
                    qt, kt, _ = st["qkv"]
                    m = p % 8
                    pts = {}
                    for i in range(4):
                        w0 = 128 * i  # valid t-cols are [w0, 512)
                        wS = min(w0, 256)  # keep moving dim >= 256 (f32r)
                        for j in (2 * m, 2 * m + 1):
                            off = 64 * (j % 2)
                            ps = psB.tile([128, 512], F32, tag="ps", bufs=4,
                                          name=f"ps{i}{j % 2}")
                            nc.tensor.matmul(
                                ps[:, wS:512],
                                qt[m][off:off + 64, w0:w0 + 128],
                                kt[m][off:off + 64, wS:512],
                                start=True, stop=False)
                            # causal mask on the PE: accumulate triT^T @ I
                            # into the diagonal block (bf16 operands: mask
                            # values are bf16-exact, full rate at any free
                            # dim); exp/PV only consume [w0:512)
                            nc.tensor.matmul(
                                ps[:, w0:w0 + 128], triT_sb[:], ident_sb[:],
                                start=False, stop=True, skip_group_check=True)
                            pt = apool.tile([128, 512], F32R, tag="pt", bufs=18,
                                            name=f"pt{i}{j % 2}")
                            nc.scalar.activation(pt[:, w0:512], ps[:, w0:512], AF.Exp)
                            pts[(j % 2, i)] = pt
                    st["pts"][p] = pts

                def pv_stage(p, st):
                    """PV accumulation for global pair p."""
                    _, _, vp = st["qkv_of"][p]
                    m = p % 8
                    pts = st["pts"].pop(p)
                    pos = {}
                    for jj in (0, 1):
                        j = 2 * m + jj
                        po = psB.tile([65, 512], F32, tag="po", bufs=2, name=f"po{jj}")
                        pos[jj] = po
                        for i in range(4):
                            w0 = 128 * i
                            nc.tensor.matmul(
                                po[0:65, w0:512],
                                vp[i][:, 65 * j:65 * (j + 1)],
                                pts[(jj, i)][:, w0:512],
                                start=(i == 0), stop=(i == 3), skip_group_check=True)
                    st["pos"][p] = pos

                def norm_stage(p, st):
                    """Normalize pair p into its oT tiles (one slot late so
                    the DVE FIFO never heads-of-line blocks on PV)."""
                    m = p % 8
                    oT = st["oT_of"][p]
                    pos = st["pos"].pop(p)
                    for jj in (0, 1):
                        off = 64 * jj
                        rs = apool.tile([1, 512], F32R, tag="rs", bufs=4, name="rs")
                        with nc.allow_low_precision(reason="f32r softmax recip"):
                            nc.vector.reciprocal(rs[:], pos[jj][64:65, :])
                        rb = apool.tile([64, 512], F32R, tag="rb", bufs=4, name="rb")
                        nc.gpsimd.partition_broadcast(rb[:], rs[:])
                        nc.vector.tensor_mul(oT[m][off:off + 64, :],
                                             pos[jj][0:64, :], rb[:])

                def op_tile(pg, st):
                    """One (tt, nn) out_proj tile for global index pg."""
                    b_op, m = divmod(pg, 8)
                    oT = st["oT_b"][b_op]
                    r0 = 512 * b_op
                    tt, nn = m % 4, m // 4
                    pf = psB.tile([128, 512], F32, tag="pf", bufs=2)
                    for k in range(NKT):
                        nc.tensor.matmul(
                            pf[:], oT[k][:, 128 * tt:128 * (tt + 1)],
                            wo_sb[k][:, 512 * nn:512 * (nn + 1)],
                            start=(k == 0), stop=(k == NKT - 1))
                    os_ = apool.tile([128, 512], F32, tag="os", bufs=3,
                                     name=f"os{tt}{nn}")
                    nc.vector.tensor_copy(os_[:], pf[:])
                    nc.sync.dma_start(
                        out[r0 + 128 * tt:r0 + 128 * (tt + 1),
                            512 * nn:512 * (nn + 1)], os_[:])

                NP = BN * (H // 2)  # 32 global pairs
                st = {"pts": {}, "pos": {}, "qkv_of": {}, "oT_of": {},
                      "oT_b": {}, "qkv": None}
                for p in range(NP + 10):
                    b, m = divmod(p, 8)
                    if p < NP:
                        if m == 0:
                            st["qkv"] = qkv_loads(b)
                            st["oT_b"][b] = [
                                apool.tile([128, 512], F32R, tag=f"oT{e}", bufs=2,
                                           name=f"oT{e}") for e in range(NKT)]
                        st["qkv_of"][p] = st["qkv"]
                        st["oT_of"][p] = st["oT_b"][b]
                        smme(p, st)
                    if p - 10 >= 0 and p - 10 < NP:
                        op_tile(p - 10, st)
                    if p - 1 >= 0 and p - 1 < NP:
                        pv_stage(p - 1, st)
                    if p - 2 >= 0 and p - 2 < NP:
                        norm_stage(p - 2, st)
            qkpool_cm.__exit__(None, None, None)

    nc.compile()
    return nc


def _ensure_built(with_qk_bias: bool, repeat: int = 1):
    key = (with_qk_bias, repeat)
    if key not in _CACHE:
        _CACHE[key] = _build(with_qk_bias, repeat)
    return _CACHE[key]


def _prepare(x, Wi, bi, Wk, bk, Wq, bq, Wv, bv, Wo, bo):
    """Host-side prep: returns (in_maps, out_const, with_qk_bias)."""
    x, Wi, bi = np.asarray(x, np.float32), np.asarray(Wi, np.float32), np.asarray(bi, np.float32)
    Wk, bk = np.asarray(Wk, np.float32), np.asarray(bk, np.float32)
    Wq, bq = np.asarray(Wq, np.float32), np.asarray(bq, np.float32)
    Wv, bv = np.asarray(Wv, np.float32), np.asarray(bv, np.float32)
    Wo, bo = np.asarray(Wo, np.float32), np.asarray(bo, np.float32)

    # flatten head-stacked weights: col f = h*HD + e
    wq_f = np.ascontiguousarray(Wq.transpose(1, 0, 2).reshape(D, D))
    wk_f = np.ascontiguousarray(Wk.transpose(1, 0, 2).reshape(D, D))
    wv_f = np.ascontiguousarray(Wv.transpose(1, 0, 2).reshape(D, D))
    # fold the in_proj into the q/k/v weights (h is only consumed by them)
    Wi64 = Wi.astype(np.float64)
    wq_c = (Wi64 @ wq_f.astype(np.float64)).astype(np.float32)
    wk_c = (Wi64 @ wk_f.astype(np.float64)).astype(np.float32)
    wv_c = (Wi64 @ wv_f.astype(np.float64)).astype(np.float32)
    # fold bi through the qkv projections; fold bv through out_proj
    bq_fold = (bi @ wq_f + bq.reshape(-1)).astype(np.float32)
    bk_fold = (bi @ wk_f + bk.reshape(-1)).astype(np.float32)
    bv_fold = (bi @ wv_f + bv.reshape(-1)).astype(np.float32)
    out_const = (bv_fold @ Wo + bo).astype(np.float32)  # added host-side

    with_qk_bias = bool(np.any(bq_fold) or np.any(bk_fold))

    # additive causal mask for the [s,t] diagonal block: -60000 where s > t
    tri_add = ((np.triu(np.ones((128, 128))) - 1.0) * -MASK_NEG).astype(np.float32)
    onesc = np.ones((128, H), np.float32)

    shared = {"wq": wq_c, "wk": wk_c, "wv": wv_c, "wo": Wo,
              "triT": np.ascontiguousarray(tri_add.T),
              "ident": np.eye(128, dtype=np.float32),
              "onesr": np.ones((1, 128), np.float32),
              "negrow": np.full((1, 128), MASK_NEG, np.float32),
              "onesc": onesc}
    if with_qk_bias:
        shared["bq2"] = np.ascontiguousarray(bq_fold.reshape(NKT, 128).T)
        shared["bk2"] = np.ascontiguousarray(bk_fold.reshape(NKT, 128).T)

    in_maps = []
    for c in range(NCORES):
        xs = x[BN * c:BN * (c + 1)].reshape(TOK, D)
        m = dict(shared)
        m["xT"] = np.ascontiguousarray(xs.T)
        in_maps.append(m)
    return in_maps, out_const, with_qk_bias


def kernel(x, Wi, bi, Wk, bk, Wq, bq, Wv, bv, Wo, bo):
    in_maps, out_const, with_qk_bias = _prepare(
        x, Wi, bi, Wk, bk, Wq, bq, Wv, bv, Wo, bo)
    nc = _ensure_built(with_qk_bias)
    res = bass_utils.run_bass_kernel_spmd(nc, in_maps, core_ids=list(range(NCORES)))
    outs = [res.results[c]["out"] for c in range(NCORES)]
    full = np.concatenate(outs, axis=0).reshape(B, T, D)
    full += out_const[None, None, :]
    return full
